# revision 2
# baseline (speedup 1.0000x reference)
"""Trainium2 Bass kernel v3 for nn_CINComp.

out[b,o,d] = sum_{i,j} W[o, i*64+j] * feature[b,i,d] * base[b,j,d] + bias[o]
Data-parallel over batch B=1024 across 8 cores (BLOC=128 batches/core).

Per-core pipeline (bf16 datapath, fp32 PSUM/out):
  - ij=4096 -> 32 K-chunks of 128 = (i-pair x 64 j)
  - F-row (2c+m) of group g lives at partition 64m+8g, chunk-slot c along
    the free dim. A per-group fixed selector S_g (K=128, full-array
    stationary like the baseline's proven pattern) broadcasts chunk c's two
    F-rows to the partition halves: fbc = S_g^T @ ftp[:, c, :].
  - ScalarE casts fbc (fp32 PSUM) -> bf16 SBUF (enables DVE 2x)
  - DVE multiplies gt2-slice x fbc16 -> P chunk-pair (bf16, 2x mode)
  - PE contracts W^T-chunk (bf16, FWL) @ P accumulating out[o,(b,d)] in PSUM
  - DVE adds bias during PSUM->SBUF copy, DMA out (fp32)
"""

import os
import numpy as np
import ml_dtypes

import concourse.bass as bass
import concourse.mybir as mybir
import concourse.tile as tile
from concourse.bass import ts
from concourse.bass_utils import run_bass_kernel_spmd

def _strip_self_waits(nc: bass.Bass) -> None:
    """Transitively-minimal semaphore waits (see kernel.py for the idea).

    This version also treats semaphores updated from MORE THAN ONE FIFO
    (engine or DMA queue) as unstrippable: their update order is not
    program order, so their thresholds can't be resolved to a unique
    producing instruction.
    """
    UPD = ("sem-inc", "sem-add-imm")
    insts = [i for bb in nc.m.functions[0].blocks for i in bb.instructions]

    bad_sems = set()
    for i in insts:
        si = getattr(i, "sync_info", None)
        if si is None:
            continue
        for u in si.on_update:
            if u.sync_type != "semaphore" or u.update_mode not in UPD:
                bad_sems.add(u.id)

    def fifo_of(i):
        si = i.sync_info
        eng = str(getattr(i, "engine", None))
        if type(i).__name__ == "InstDMACopy" and si is not None:
            for u in si.on_update:
                if u.sync_type == "semaphore" and u.update_mode in UPD:
                    return ("q", u.id)
        return ("e", eng)

    sem_fifos: dict = {}
    for i in insts:
        si = getattr(i, "sync_info", None)
        if si is None:
            continue
        for u in si.on_update:
            if u.sync_type == "semaphore" and u.update_mode in UPD:
                sem_fifos.setdefault(u.id, set()).add(fifo_of(i))
    for s, fs in sem_fifos.items():
        if len(fs) > 1:
            bad_sems.add(s)

    cum: dict = {}
    event: dict = {}
    fifo_pred: dict = {}
    last_in_fifo: dict = {}
    metas = []
    for idx, i in enumerate(insts):
        si = getattr(i, "sync_info", None)
        f = fifo_of(i)
        fifo_pred[idx] = last_in_fifo.get(f)
        last_in_fifo[f] = idx
        ups = []
        if si is not None:
            for u in si.on_update:
                if u.sync_type == "semaphore" and u.update_mode in UPD:
                    cum[u.id] = cum.get(u.id, 0) + u.update_value
                    event[(u.id, cum[u.id])] = idx
                    ups.append((u.id, cum[u.id]))
        metas.append((si, ups))

    def resolve(sem, k):
        v = k
        while (sem, v) not in event:
            v += 1
            if v > cum.get(sem, 0):
                return None
        return event[(sem, v)]

    cvc: list = [None] * len(insts)

    def get_cvc(idx):
        if cvc[idx] is not None:
            return cvc[idx]
        stack = [idx]
        while stack:
            j = stack[-1]
            if cvc[j] is not None:
                stack.pop()
                continue
            si, ups = metas[j]
            deps = []
            p = fifo_pred[j]
            if p is not None:
                deps.append(p)
            if si is not None:
                for w in si.on_wait:
                    if (
                        w.sync_type == "semaphore"
                        and w.wait_mode == "sem-ge-imm"
                        and w.id not in bad_sems
                    ):
                        e = resolve(w.id, w.wait_value)
                        if e is not None and e != j:
                            deps.append(e)
            pending = [d for d in deps if cvc[d] is None]
            if pending:
                stack.extend(pending)
                continue
            stack.pop()
            vc: dict = {}
            for d in deps:
                for s, v in cvc[d].items():
                    if vc.get(s, 0) < v:
                        vc[s] = v
            if si is not None:
                for w in si.on_wait:
                    if (
                        w.sync_type == "semaphore"
                        and w.wait_mode == "sem-ge-imm"
                        and w.id not in bad_sems
                    ):
                        if vc.get(w.id, 0) < w.wait_value:
                            vc[w.id] = w.wait_value
            for s, v in ups:
                if vc.get(s, 0) < v:
                    vc[s] = v
            cvc[j] = vc
        return cvc[idx]

    for idx, i in enumerate(insts):
        si, _ups = metas[idx]
        if si is None or not si.on_wait:
            continue
        base: dict = {}
        p = fifo_pred[idx]
        if p is not None:
            base = dict(get_cvc(p))
        sem_waits = [
            w
            for w in si.on_wait
            if w.sync_type == "semaphore"
            and w.wait_mode == "sem-ge-imm"
            and w.id not in bad_sems
        ]
        other = [w for w in si.on_wait if w not in sem_waits]

        def strength(w):
            e = resolve(w.id, w.wait_value)
            return len(get_cvc(e)) if e is not None else 0

        sem_waits.sort(key=strength, reverse=True)

        def wait_cvc(w):
            e = resolve(w.id, w.wait_value)
            vc = dict(get_cvc(e)) if e is not None else {}
            if vc.get(w.id, 0) < w.wait_value:
                vc[w.id] = w.wait_value
            return vc

        kept = sem_waits[:]
        changed = True
        while changed:
            changed = False
            for w in kept:
                cover = dict(base)
                for w2 in kept:
                    if w2 is w:
                        continue
                    for s, v in wait_cvc(w2).items():
                        if cover.get(s, 0) < v:
                            cover[s] = v
                if cover.get(w.id, 0) >= w.wait_value:
                    kept.remove(w)
                    changed = True
                    break
        if len(kept) + len(other) != len(si.on_wait):
            si.on_wait = other + kept


B, HK, H0, D, O = 1024, 64, 64, 32, 128
NCORES = 8
BLOC = B // NCORES          # 128 batches per core
GROUPS = 8
GB = BLOC // GROUPS         # 16 batches per group
N = GB * D                  # 512
NCHUNK = 32                 # K chunks of 128 over ij=4096
F32 = mybir.dt.float32
BF16 = mybir.dt.bfloat16

_CACHE = {}


def _build_nc(strip: bool = True) -> bass.Bass:
    nc = bass.Bass()
    # ftp dram row q=8m+g -> sbuf partition q; free [chunk 32, 512]
    ftp = nc.dram_tensor("ftp", [16, NCHUNK * N], BF16, kind="ExternalInput")
    gt2 = nc.dram_tensor("gt2", [128, BLOC * D], BF16, kind="ExternalInput")
    wt = nc.dram_tensor("wt", [128, NCHUNK * 128], BF16, kind="ExternalInput")
    # selm dram row q=8m+g holds its full [GROUPS, 128] selector slice:
    # nonzero only at [g, :]: (mm < 64) == (m == 0)
    selm = nc.dram_tensor("selm", [16, GROUPS * 128], BF16,
                          kind="ExternalInput")
    bias = nc.dram_tensor("bias", [128, 1], F32, kind="ExternalInput")
    out = nc.dram_tensor("out", [128, BLOC * D], F32, kind="ExternalOutput")

    cast_mode = os.environ.get("V3_CAST", "mix14")

    with tile.TileContext(nc) as tc:
        with (
            tc.tile_pool(name="resident", bufs=1) as res,
            tc.tile_pool(name="fb16", bufs=4) as fpool16,
            tc.tile_pool(name="p", bufs=6) as ppool,
            tc.tile_pool(name="osb", bufs=4) as opool,
            tc.tile_pool(name="fbc", bufs=3, space="PSUM") as fpool,
            tc.tile_pool(name="acc", bufs=2, space="PSUM") as apool,
        ):
            ftp_sb = res.tile([128, NCHUNK, N], BF16)
            gt2_sb = res.tile([128, BLOC * D], BF16)
            wt_sb = res.tile([128, NCHUNK * 128], BF16)
            sel_sb = res.tile([128, GROUPS, 128], BF16)
            bias_sb = res.tile([128, 1], F32)

            # rows not covered by the DMAs below are multiplied by zero
            # selector weights; they only need to be NaN-free.
            nc.gpsimd.memset(sel_sb[:].bitcast(F32), 0.0)
            nc.gpsimd.memset(ftp_sb[:].bitcast(F32), 0.0)

            nc.sync.dma_start(out=bias_sb[:], in_=bias[:])
            nc.sync.dma_start(out=ftp_sb[0:16, :, :],
                              in_=ftp[:, :].rearrange(
                                  "p (c n) -> p c n", c=NCHUNK))
            nc.sync.dma_start(out=sel_sb[0:16, :, :],
                              in_=selm[:, :].rearrange(
                                  "p (g n) -> p g n", g=GROUPS))
            # touch each DMA's region: move its queue-sem dep onto the DVE
            # clock (instruction structs accept only one wait)
            nc.vector.tensor_copy(ftp_sb[0:1, 0, 0:1], ftp_sb[0:1, 0, 0:1])
            nc.vector.tensor_copy(sel_sb[0:1, 0, 0:1], sel_sb[0:1, 0, 0:1])


            Q = BLOC * D // 4
            for q in range(4):
                nc.sync.dma_start(out=gt2_sb[:, ts(q, Q)], in_=gt2[:, ts(q, Q)])
                nc.sync.dma_start(out=wt_sb[:, ts(q, Q)], in_=wt[:, ts(q, Q)])
                for tsb in (gt2_sb, wt_sb):
                    nc.vector.tensor_copy(tsb[0:1, q * Q:q * Q + 1],
                                          tsb[0:1, q * Q:q * Q + 1])
            nc.vector.tensor_copy(bias_sb[0:1, 0:1], bias_sb[0:1, 0:1])
            tiny = res.tile([128, 1], F32)
            nc.vector.tensor_copy(tiny[:], bias_sb[:, 0:1])

            n_act = (16 if cast_mode == "act"
                     else int(cast_mode[3:]) if cast_mode.startswith("mix")
                     else 0)
            n_dve = 16 - n_act
            dve_ts = {int((k + 0.5) * 16 / n_dve) for k in range(n_dve)} \
                if n_dve else set()
            for g in range(GROUPS):
                acc = apool.tile([128, N], F32, tag="acc")
                for t2 in range(NCHUNK // 4):
                    # 4 chunks per iteration: 2 fbc pair-tiles -> one fb16
                    fb16 = fpool16.tile([128, 4, N], BF16, tag="fb16")
                    for half in range(2):
                        t = 2 * t2 + half
                        fbc = fpool.tile([128, 2, N], F32, tag="fbc")
                        for cc in range(2):
                            c = 2 * t + cc
                            nc.tensor.matmul(fbc[:, cc, :], sel_sb[:, g, :],
                                             ftp_sb[:, c, :],
                                             start=True, stop=True)
                        if t not in dve_ts:
                            nc.scalar.copy(fb16[:, 2 * half:2 * half + 2, :],
                                           fbc[:])
                        else:
                            nc.vector.tensor_copy(
                                fb16[:, 2 * half:2 * half + 2, :], fbc[:])
                    gview = gt2_sb[:, ts(g, N)][:, None, :].to_broadcast(
                        (128, 4, N))
                    p = ppool.tile([128, 4, N], BF16, tag="pd")
                    nc.vector.tensor_mul(p[:], gview, fb16[:])
                    for j in range(4):
                        c = 4 * t2 + j
                        nc.tensor.matmul(acc[:], wt_sb[:, ts(c, 128)],
                                         p[:, j, :], start=(c == 0),
                                         stop=(c == NCHUNK - 1))

                osb = opool.tile([128, N], F32, tag="osb")
                nc.vector.tensor_scalar(osb[:], acc[:], bias_sb[:, 0:1],
                                        None, mybir.AluOpType.add)
                nc.sync.dma_start(out=out[:, ts(g, N)], in_=osb[:])
                nc.vector.tensor_copy(osb[0:1, 0:1], tiny[0:1, 0:1])

    if strip:
        _strip_self_waits(nc)
    return nc


def _get_nc() -> bass.Bass:
    key = ("nc", os.environ.get("V3_CAST", "act"))
    if key not in _CACHE:
        _CACHE[key] = _build_nc()
    return _CACHE[key]


def _prep_core_inputs(feature, base, W, b, ci):
    bsl = slice(ci * BLOC, (ci + 1) * BLOC)
    F = np.ascontiguousarray(feature[bsl], np.float32)  # (128, 64, 32)
    G = np.ascontiguousarray(base[bsl], np.float32)     # (128, 64, 32)

    Gt = np.transpose(G, (1, 0, 2))
    gt2 = np.concatenate([Gt, Gt], 0).reshape(128, BLOC * D)

    Ft = np.transpose(F, (1, 0, 2)).reshape(HK, BLOC * D)  # (i, b*d)
    # ftp[8m+g, c, :] = Ft[2c+m, g*N:(g+1)*N]
    FtR = Ft.reshape(NCHUNK, 2, GROUPS, N)      # (c, m, g, n)
    ftp = np.transpose(FtR, (1, 2, 0, 3)).reshape(16, NCHUNK * N)

    wt = np.transpose(W.reshape(O, NCHUNK, 128), (2, 1, 0)).reshape(
        128, NCHUNK * 128)

    # selm[8m+g, g, mm] = 1 iff (mm < 64) == (m == 0)
    selm = np.zeros((16, GROUPS, 128), np.float32)
    for g in range(GROUPS):
        selm[g, g, 0:64] = 1.0
        selm[8 + g, g, 64:128] = 1.0
    selm = selm.reshape(16, GROUPS * 128)

    bf = ml_dtypes.bfloat16
    return {
        "ftp": np.ascontiguousarray(ftp).astype(bf),
        "gt2": gt2.astype(bf),
        "wt": np.ascontiguousarray(wt).astype(bf),
        "selm": selm.astype(bf),
        "bias": np.ascontiguousarray(b, np.float32).reshape(128, 1),
    }


def run(feature, base, W, b, **spmd_kwargs):
    ncores = int(os.environ.get("V3_CORES", NCORES))
    nc = _get_nc()
    in_maps = [_prep_core_inputs(feature, base, W, b, ci) for ci in range(ncores)]
    res = run_bass_kernel_spmd(nc, in_maps, list(range(ncores)), **spmd_kwargs)
    outs = []
    for ci in range(ncores):
        o = res.results[ci]["out"].reshape(O, BLOC, D)
        outs.append(np.transpose(o, (1, 0, 2)))
    full = np.concatenate(outs, 0)
    if ncores < NCORES:
        full = np.concatenate([full] * (NCORES // ncores), 0)
    return full, res


def kernel(feature, base, W, b):
    full, _ = run(feature, base, W, b)
    return full
